# revision 13
# baseline (speedup 1.0000x reference)
"""Trainium2 Bass kernel for nn_MultiHeadAttention_59227599012491.

Reference computation (per batch b):
    xf = x[b].reshape(S, 256)
    q  = softplus(xf @ Wq.T + bq);  k = softplus(xf @ Wk.T + bk)
    v  = xf @ Wv.T + bv
    weight = q @ k.T            (no softmax!)
    result = weight @ v
    out    = result @ Wo.T + bo

Because there is no softmax, attention is associative:
    result = (q @ k.T) @ v = q @ (k.T @ v) = q @ G,   G: [256, 256]
    out    = q @ (G @ Wo.T) + bo = q @ M + bo
so the S x S score matrix never needs to be materialized. Per-core work
drops to a handful of [*, 256] x [256, 256] matmuls; the kernel is
memory-bound on streaming x in and out once.

Sharding: B=4 batches x 2 query-halves -> 8 cores, no collectives.
Each core computes k/v/G/M for its whole batch (cheap, duplicated
within a pair) and the output rows for its half of the queries.

Layouts (PE computes out = lhsT.T @ rhs, contracting partition dim):
    xbT  [256, 4096]  x[b] transposed on host (queries first SQ cols)
    qT   [256, 2048]  lhsT = WqT tile, rhs = xbT     (softplus via ACT,
                      bias per-partition, fused into the Exp pass)
    kv   [4096, 512]  k and v fused: rhs = [WkT | WvT], one stationary
                      xbT tile per row tile serves both. +[bk|bv] via a
                      single DVE add; softplus on the k half in-place
                      (ACT Exp then Ln(1+t), batched over tile pairs)
    GT   [256, 256]   GT[d,e] = sum_s v[s,d] k[s,e]: lhsT = v t, rhs = k t
    M    [256, 256]   M[e,do] = sum_d GT[d,e] WoT[d,do]: lhsT = GT, rhs = WoT
    out  [2048, 256]  lhsT = qT tile, rhs = M        (bias via DVE add)

float32r streams at 1 cycle/row on the PE (vs 4 for exact float32);
hardware requires both matmul operands f32r, even free-dim counts, and
8B-aligned PSUM destinations, so the tiny K=1 bias/broadcast matmuls
stay plain fp32.

The activation-table pass is steered to `natural_log_exp_and_others`
(the only set holding Exp AND Ln) so the ACT engine loads its PWP table
once instead of reloading per activation (24 loads ~= 30us saved).
"""

import numpy as np

S = 4096
SQ = 2048  # query rows per core
D = 256
P = 128
IT = D // P  # 2 input-dim tiles
DT = D // P  # 2 d-model tiles
NS = S // P  # 32 sequence tiles
BLK = 512  # free-dim block for qT
N_CORES = 8

MM_DTYPE_NAME = "float16"

_CACHE = {}


def _patched_act_tables(orig_fn):
    def patched(arch):
        tabs = orig_fn(arch)
        return {
            name: (s if name == "natural_log_exp_and_others" else set())
            for name, s in tabs.items()
        }

    return patched


def _build_nc():
    import concourse.bacc as bacc
    import concourse.mybir as mybir
    import concourse.tile as tile

    FP = mybir.dt.float32
    FR = getattr(mybir.dt, MM_DTYPE_NAME)
    AF = mybir.ActivationFunctionType
    ADD = mybir.AluOpType.add

    nc = bacc.Bacc("TRN2", target_bir_lowering=False, debug=False, num_devices=1)

    xbT_d = nc.declare_dram_parameter("xbT", [D, S], FR, isOutput=False)
    wqT_d = nc.declare_dram_parameter("wqT", [D, D], FR, isOutput=False)
    wkvT_d = nc.declare_dram_parameter("wkvT", [D, 2 * D], FR, isOutput=False)
    woT_d = nc.declare_dram_parameter("woT", [D, D], FR, isOutput=False)
    bq_d = nc.declare_dram_parameter("bq", [1, D], FP, isOutput=False)
    bkv_d = nc.declare_dram_parameter("bkv", [1, 4 * D], FP, isOutput=False)
    bo2_d = nc.declare_dram_parameter("bo2", [1, 2 * D], FP, isOutput=False)
    out_d = nc.declare_dram_parameter("out", [SQ, D], FP, isOutput=True)

    def mm(psum, lhsT, rhs, start, stop):
        nc.tensor.matmul(psum, lhsT, rhs, start=start, stop=stop)

    with tile.TileContext(nc) as tc:
        with (
            tc.tile_pool(name="w", bufs=1) as wpool,
            tc.tile_pool(name="big", bufs=1) as big,
            tc.tile_pool(name="tmp", bufs=4) as tpool,
            tc.tile_pool(name="ob", bufs=4) as opool,
            tc.tile_pool(name="psQ", bufs=2, space="PSUM") as psQ,
            tc.tile_pool(name="psKV", bufs=2, space="PSUM") as psKV,
            tc.tile_pool(name="psG", bufs=2, space="PSUM") as psG,
        ):
            # --- loads; ordered so the kv pipeline starts ASAP:
            # wkv + first xbT chunk first, bias tiles via DMA tricks
            # (partition-strided for bqT, DRAM-broadcast for the bias
            # rows) so the PE never touches bias setup ---
            wq_sb = wpool.tile([P, IT, D], FR, tag="wq")
            wo_sb = wpool.tile([P, IT, D], FR, tag="wo")
            wkv_sb = wpool.tile([P, IT, 2 * D], FR, tag="wkv")
            xbT_sb = big.tile([P, IT, S], FR, tag="xbT")
            bqT = wpool.tile([P, DT], FP, tag="bqT")
            bc_bkv = wpool.tile([P, 4 * D], FP, tag="bc_bkv")
            bc_bo2 = wpool.tile([P, 2 * D], FP, tag="bc_bo2")
            b_bc = {"bkv": bc_bkv, "bo2": bc_bo2}
            for it in range(IT):
                nc.sync.dma_start(wkv_sb[:, it, :], wkvT_d.ap()[it * P : (it + 1) * P, :])
            nc.sync.dma_start(
                b_bc["bkv"][:, :], bkv_d.ap()[0:1, :].broadcast_to([P, 4 * D])
            )
            nc.sync.dma_start(
                xbT_sb[:, :, 0:1024],
                xbT_d.ap()[:, 0:1024].rearrange("(it p) s -> p it s", p=P),
            )
            nc.sync.dma_start(
                xbT_sb[:, :, 1024:2048],
                xbT_d.ap()[:, 1024:2048].rearrange("(it p) s -> p it s", p=P),
            )
            for dt in range(DT):
                nc.sync.dma_start(
                    bqT[:, dt : dt + 1],
                    bq_d.ap()[0:1, dt * P : (dt + 1) * P].rearrange("a (p w) -> (a p) w", w=1),
                )
            for it in range(IT):
                nc.sync.dma_start(wq_sb[:, it, :], wqT_d.ap()[it * P : (it + 1) * P, :])
            nc.sync.dma_start(
                xbT_sb[:, :, 2048:3072],
                xbT_d.ap()[:, 2048:3072].rearrange("(it p) s -> p it s", p=P),
            )
            nc.sync.dma_start(
                b_bc["bo2"][:, :], bo2_d.ap()[0:1, :].broadcast_to([P, 2 * D])
            )
            nc.sync.dma_start(
                xbT_sb[:, :, 3072:4096],
                xbT_d.ap()[:, 3072:4096].rearrange("(it p) s -> p it s", p=P),
            )
            for it in range(IT):
                nc.sync.dma_start(wo_sb[:, it, :], woT_d.ap()[it * P : (it + 1) * P, :])

            # kv planes: kv_sb[:, 0, t, :] = k (post-softplus),
            #            kv_sb[:, 1, t, :] = v
            kv_sb = big.tile([P, 2, NS, D], FR, tag="kv")
            qT_sb = big.tile([P, DT, SQ], FR, tag="qT")
            GT_sb = wpool.tile([P, DT, D], FR, tag="GT")
            M_sb = wpool.tile([P, DT, D], FR, tag="M")

            # --- kv = x [WkT | WvT] + [bk | bv], two row tiles per
            # 2-bank psum tile so the DVE add runs as one [128,1024] op ---
            for pr in range(NS // 2):
                ps = psKV.tile([P, 2, 2 * D], FP, tag="psKV")
                for j in range(2):
                    t = 2 * pr + j
                    ts = slice(t * P, (t + 1) * P)
                    for it in range(IT):
                        mm(ps[:, j, :], xbT_sb[:, it, ts], wkv_sb[:, it, :], it == 0, it == IT - 1)
                nc.vector.tensor_tensor(
                    kv_sb[:, :, 2 * pr : 2 * pr + 2, :],
                    ps[:, :, :].rearrange("p j (pl d) -> p pl j d", pl=2),
                    b_bc["bkv"][:, :].rearrange("p (j pl d) -> p pl j d", j=2, pl=2),
                    op=ADD,
                )
                # softplus on contiguous k-plane runs: 2,2,4,4,... tiles so
                # the ACT chain starts as early as possible
                t = 2 * pr + 1
                if t in (1, 3) or (t > 3 and t % 4 == 3):
                    n = 2 if t <= 3 else 4
                    tt = slice(t - n + 1, t + 1)
                    tmp = tpool.tile([P, 4, D], FP, tag="tmpk")
                    nc.scalar.activation(tmp[:, 0:n, :], kv_sb[:, 0, tt, :], AF.Exp)
                    nc.scalar.activation(kv_sb[:, 0, tt, :], tmp[:, 0:n, :], AF.Ln, bias=1.0)

            # --- qT = softplus(Wq x^T + bq), transposed layout [e, sq] ---
            for dt in range(DT):
                ds = slice(dt * P, (dt + 1) * P)
                for blk in range(SQ // BLK):
                    ss = slice(blk * BLK, (blk + 1) * BLK)
                    ps = psQ.tile([P, BLK], FP, tag="psQ")
                    for it in range(IT):
                        mm(ps[:, :], wq_sb[:, it, ds], xbT_sb[:, it, ss], it == 0, it == IT - 1)
                    tmp = tpool.tile([P, BLK], FP, tag="tmpq")
                    nc.scalar.activation(tmp[:, :], ps[:, :], AF.Exp, bias=bqT[:, dt : dt + 1])
                    nc.scalar.activation(qT_sb[:, dt, ss], tmp[:, :], AF.Ln, bias=1.0)

            # --- GT[d, e] = sum_s v[s, d] k[s, e] ---
            for dt in range(DT):
                vs = slice(dt * P, (dt + 1) * P)
                ps = psG.tile([P, D], FP, tag="psG")
                for t in range(NS):
                    mm(ps[:, :], kv_sb[:, 1, t, vs], kv_sb[:, 0, t, :], t == 0, t == NS - 1)
                nc.vector.tensor_copy(GT_sb[:, dt, :], ps[:, :])

            # --- M[e, do] = sum_d GT[d, e] WoT[d, do] ---
            for et in range(DT):
                es = slice(et * P, (et + 1) * P)
                ps = psG.tile([P, D], FP, tag="psG")
                for dt in range(DT):
                    mm(ps[:, :], GT_sb[:, dt, es], wo_sb[:, dt, :], dt == 0, dt == DT - 1)
                nc.vector.tensor_copy(M_sb[:, et, :], ps[:, :])

            # --- out[sq, do] = sum_e q[sq, e] M[e, do] + bo, pairs ---
            for pr in range(SQ // (2 * P)):
                ps = psQ.tile([P, 2, D], FP, tag="psQ")
                for j in range(2):
                    ss = slice((2 * pr + j) * P, (2 * pr + j + 1) * P)
                    for et in range(DT):
                        mm(ps[:, j, :], qT_sb[:, et, ss], M_sb[:, et, :], et == 0, et == DT - 1)
                ob = opool.tile([P, 2, D], FP, tag="ob")
                nc.vector.tensor_tensor(
                    ob[:, :, :], ps[:, :, :],
                    b_bc["bo2"][:, :].rearrange("p (j d) -> p j d", j=2), op=ADD,
                )
                nc.sync.dma_start(
                    out_d.ap()[2 * pr * P : (2 * pr + 2) * P, :].rearrange(
                        "(j p) d -> p j d", p=P
                    ),
                    ob[:, :, :],
                )

    # Steer the activation-table pass: only natural_log_exp_and_others
    # (set 6) contains both Exp and Ln, so one PWP table load suffices.
    import concourse.hw_specs as hw_specs

    orig = bacc.get_activation_tables
    bacc.get_activation_tables = _patched_act_tables(hw_specs.get_activation_tables)
    try:
        nc.compile()
    finally:
        bacc.get_activation_tables = orig
    return nc


def _get_nc():
    nc = _CACHE.get("nc")
    if nc is None:
        nc = _build_nc()
        _CACHE["nc"] = nc
    return nc


def make_in_maps(x, Wq, bq, Wk, bk, Wv, bv, Wo, bo):
    B = x.shape[0]
    mmnp = np.float16 if MM_DTYPE_NAME == "float16" else np.float32
    xf = np.asarray(x, dtype=np.float32).reshape(B, S, D)
    xfT = np.ascontiguousarray(xf.transpose(0, 2, 1).astype(mmnp))  # [B, 256, 4096]
    shared = {
        "wqT": np.ascontiguousarray(np.asarray(Wq, mmnp).T),
        "wkvT": np.ascontiguousarray(
            np.hstack([np.asarray(Wk, mmnp).T, np.asarray(Wv, mmnp).T])
        ),
        "woT": np.ascontiguousarray(np.asarray(Wo, mmnp).T),
        "bq": np.asarray(bq, np.float32).reshape(1, D),
        "bkv": np.tile(
            np.concatenate([np.asarray(bk, np.float32), np.asarray(bv, np.float32)]), 2
        ).reshape(1, 4 * D),
        "bo2": np.tile(np.asarray(bo, np.float32), 2).reshape(1, 2 * D),
    }
    in_maps = []
    for c in range(N_CORES):
        b, h = divmod(c, 2)
        xT = xfT[b]
        if h == 1:
            xT = np.concatenate([xT[:, SQ:], xT[:, :SQ]], axis=1)
        in_maps.append({"xbT": np.ascontiguousarray(xT), **shared})
    return in_maps


def assemble_out(results, x_shape):
    B, S_, H, W = x_shape
    out = np.empty((B, S_, D), np.float32)
    for c in range(N_CORES):
        b, h = divmod(c, 2)
        out[b, h * SQ : (h + 1) * SQ] = results[c]["out"]
    return out.reshape(B, S_, H, W)


def kernel(x, Wq, bq, Wk, bk, Wv, bv, Wo, bo, _trace=False):
    from concourse.bass_utils import run_bass_kernel_spmd

    nc = _get_nc()
    in_maps = make_in_maps(x, Wq, bq, Wk, bk, Wv, bv, Wo, bo)
    res = run_bass_kernel_spmd(nc, in_maps, list(range(N_CORES)), trace=_trace)
    out = assemble_out(res.results, x.shape)
    if _trace:
        _CACHE["last_result"] = res
    return out
